# revision 1
# baseline (speedup 1.0000x reference)
"""GeAT layer (graph attention w/ per-edge MLP scoring) on 8 Trainium2 cores.

Strategy (fully sparse — the dense [H,N,N] tensor is never materialized):
  - Directed edges (symmetric doubling of the input edge list) are deduped
    (last-write-wins, matching XLA scatter-set) and sharded by SOURCE row:
    core c owns rows [c*512, (c+1)*512), i.e. all softmax rows it outputs.
    Fully data-parallel: no cross-core communication at all.
  - Host prep is index/layout work plus parameter fusion: per-edge gathered
    embeddings emb[src], emb[dst] are shipped transposed as one
    [128, E_core] operand, edges sorted by (row-block, bond) with padding so
    every core runs the identical SPMD program. Qw/Kw/Qb/Kb are fused into
    the first MLP layer's weights (h0 = relu([emb_s emb_d] @ [[Qw@W0t],
    [Kw@W0b]] + b0')) so no separate Q/K projection stage is needed.
  - On device, per core: per-bond 2-layer MLP on raw edge embeddings (heads
    packed in pairs of 64 into the 128-wide PE array) -> leaky-relu ->
    per-(rowblock,head) max-subtracted exp -> segment softmax-aggregate
    expressed as PSUM-accumulated matmuls against an iota==srcrel 0/1 mask
    (the "scatter") with w-scaled V rows (the "gather") -> final projection.
  - The edge-embedding stream, MLP weights, scatter mask and aggregation
    operands are bf16 (exact for the 0/1 mask; the kernel is HBM-bound on
    the per-edge stream); scores and softmax weights/normalization stay
    f32/float32r. Emission is software-pipelined at half-row-block
    granularity so the PE/ACT MLP stream of one unit overlaps the
    DVE/GPSIMD softmax-aggregate tail of the previous one.
"""

import sys

sys.path.insert(0, "/opt/trn_rl_repo")

import numpy as np

N, D, H, B, HID = 4096, 64, 4, 4, 64
NEG = 0.2
C = 8            # cores
RPC = N // C     # rows per core
NRB = 4          # row blocks per core
RBS = 128        # rows per block

_cache = {}


def _host_prep(embeddings, src, dst, bond):
    emb = np.ascontiguousarray(np.asarray(embeddings, np.float32))
    src = np.asarray(src).astype(np.int64)
    dst = np.asarray(dst).astype(np.int64)
    bond = np.asarray(bond).astype(np.int64)

    s_all = np.concatenate([src, dst])
    d_all = np.concatenate([dst, src])
    b_all = np.concatenate([bond, bond])
    L = s_all.shape[0]

    # scatter-set duplicate resolution: last occurrence wins
    key = s_all * N + d_all
    order = np.argsort(key, kind="stable")
    ks = key[order]
    is_last = np.ones(L, bool)
    is_last[:-1] = ks[1:] != ks[:-1]
    alive = np.zeros(L, bool)
    alive[order[is_last]] = True

    core = s_all // RPC
    rb = (s_all % RPC) // RBS
    srel = (s_all % RBS).astype(np.float32)

    counts = np.zeros((C, NRB, B), np.int64)
    np.add.at(counts, (core[alive], rb[alive], b_all[alive]), 1)
    Lb = [int(-(-counts[:, :, b].max() // 128) * 128) for b in range(B)]
    offs = np.concatenate([[0], np.cumsum(Lb)]).astype(np.int64)
    R = int(offs[-1])
    ERUN = NRB * R
    NTILE = ERUN // 128

    xembT = np.zeros((C, 128, ERUN), np.float32)
    srcrel = np.full((C, 128, NTILE), -1.0, np.float32)
    bondslot = np.zeros((C, 128, NTILE), np.int64)
    for c in range(C):
        for r in range(NRB):
            for b in range(B):
                sel = np.where(alive & (core == c) & (rb == r) & (b_all == b))[0]
                lo = r * R + int(offs[b])
                allslots = lo + np.arange(Lb[b])
                bondslot[c, allslots % 128, allslots // 128] = b
                if len(sel) == 0:
                    continue
                slots = lo + np.arange(len(sel))
                xembT[c, 0:64, slots] = emb[s_all[sel]]
                xembT[c, 64:128, slots] = emb[d_all[sel]]
                srcrel[c, slots % 128, slots // 128] = srel[sel]
    return xembT, srcrel, bondslot, Lb, R


def _weights_prep(inp):
    f32 = np.float32
    Qw, Qb = np.asarray(inp["Qw"], f32), np.asarray(inp["Qb"], f32)
    Kw, Kb = np.asarray(inp["Kw"], f32), np.asarray(inp["Kb"], f32)
    Vw, Vb = np.asarray(inp["Vw"], f32), np.asarray(inp["Vb"], f32)
    W0, b0 = np.asarray(inp["W0"], f32), np.asarray(inp["b0"], f32)
    W1, b1 = np.asarray(inp["W1"], f32), np.asarray(inp["b1"], f32)
    W2, b2 = np.asarray(inp["W2"], f32), np.asarray(inp["b2"], f32)
    Pw, Pb = np.asarray(inp["Pw"], f32), np.asarray(inp["Pb"], f32)

    z = np.zeros((64, 64), f32)
    vwpad = np.concatenate([z, Vw], 0)                    # [128, 64]

    # fuse the Q/K projections into the first MLP layer (per bond, head)
    fw0 = np.zeros((B, H, 128, HID), f32)
    fb0 = np.zeros((B, H, HID), f32)
    for b in range(B):
        for h in range(H):
            fw0[b, h, 0:64] = Qw @ W0[b, h, 0:64]
            fw0[b, h, 64:128] = Kw @ W0[b, h, 64:128]
            fb0[b, h] = Qb @ W0[b, h, 0:64] + Kb @ W0[b, h, 64:128] + b0[b, h]

    w0all = np.zeros((128, B * 2 * 128), f32)
    w1all = np.zeros((128, B * 2 * 128), f32)
    w2all = np.zeros((128, B * 2 * 2), f32)
    b0all = np.zeros((128, B * 2), f32)
    b1all = np.zeros((128, B * 2), f32)
    b2all = np.zeros((2, B * 2), f32)
    for b in range(B):
        for pr in range(2):
            i = b * 2 + pr
            ha, hb = 2 * pr, 2 * pr + 1
            w0all[:, i * 128: i * 128 + 64] = fw0[b, ha]
            w0all[:, i * 128 + 64: (i + 1) * 128] = fw0[b, hb]
            w1all[0:64, i * 128: i * 128 + 64] = W1[b, ha]
            w1all[64:128, i * 128 + 64: (i + 1) * 128] = W1[b, hb]
            w2all[0:64, i * 2] = W2[b, ha]
            w2all[64:128, i * 2 + 1] = W2[b, hb]
            b0all[0:64, i] = fb0[b, ha]
            b0all[64:128, i] = fb0[b, hb]
            b1all[0:64, i] = b1[b, ha]
            b1all[64:128, i] = b1[b, hb]
            b2all[0, i] = b2[b, ha]
            b2all[1, i] = b2[b, hb]

    pw4 = np.zeros((64, H * 64), f32)                     # lhsT per head
    for h in range(H):
        pw4[:, h * 64:(h + 1) * 64] = Pw[h * 64:(h + 1) * 64]
    biascol = (Pb + np.tile(Vb, H) @ Pw)[:, None]         # [64, 1]

    iota = np.tile(np.arange(128, dtype=f32), (128, 1))   # [128, 128]
    id128 = np.eye(128, dtype=f32)

    return dict(vwpad=vwpad, w0all=w0all, w1all=w1all, w2all=w2all,
                b0all=b0all, b1all=b1all, b2all=b2all,
                pw4=pw4, biascol=biascol,
                iota=iota, id128=id128)


def _chunks(n, step=512):
    out = []
    s = 0
    while s < n:
        out.append((s, min(step, n - s)))
        s += step
    return out


def _build_program(Lb, R, loop=0):
    import concourse.bacc as bacc
    import concourse.tile as tile
    from concourse import mybir
    from contextlib import ExitStack

    f32 = mybir.dt.float32
    fr = mybir.dt.float32r
    bf = mybir.dt.bfloat16
    AF = mybir.ActivationFunctionType
    ALU = mybir.AluOpType

    ERUN = NRB * R
    NTILE = ERUN // 128
    TPB = R // 128
    offs = np.concatenate([[0], np.cumsum(Lb)]).astype(np.int64)
    TA = int(offs[2]) // 128           # tiles in bond group A = {0, 1}
    GRP = [(0, [0, 1], 0, TA), (1, [2, 3], TA, TPB - TA)]
    NG = len(GRP)

    # packed constant layouts (column offsets)
    CPK = {}
    o = 0
    for nm, w in [("srcrel", NTILE), ("iota", 128), ("biascol", 1)]:
        CPK[nm] = (o, w); o += w
    CPKW = o
    BPK = {}
    o = 0
    for nm, w in [("b0all", B * 2), ("b1all", B * 2)]:
        BPK[nm] = (o, w); o += w
    BPKW = o
    WBF = {}
    o = 0
    for nm, w in [("vwpad", 64), ("w2all", B * 2 * 2)]:
        WBF[nm] = (o, w); o += w
    WBFW = o
    WPK = {}
    o = 0
    for nm, w in [("pw4", H * 64), ("id128", 128), ("w2fr", B * 2 * 2)]:
        WPK[nm] = (o, w); o += w
    WPKW = o

    nc = bacc.Bacc("TRN2", target_bir_lowering=False, debug=False, num_devices=C)

    dspec = [("xembT", (128, ERUN), bf),
             ("w0b0", (128, 256), bf), ("w0r", (128, 768), bf),
             ("w1b0", (128, 256), bf), ("w1r", (128, 768), bf),
             ("bpk", (128, BPKW), f32), ("wbf", (128, WBFW), bf),
             ("wpkt", (128, WPKW), fr),
             ("cpk", (128, CPKW), f32), ("b2e", (128, NTILE * H), f32)]
    dram = {}
    for nm, shp, dt in dspec:
        dram[nm] = nc.dram_tensor(nm, list(shp), dt, kind="ExternalInput").ap()
    outT = nc.dram_tensor("outT", [64, RPC], f32, kind="ExternalOutput").ap()

    with ExitStack() as ctx:
        tc = ctx.enter_context(tile.TileContext(nc))
        constp = ctx.enter_context(tc.tile_pool(name="const", bufs=1))
        xep = ctx.enter_context(tc.tile_pool(name="xe", bufs=3))
        hidp = ctx.enter_context(tc.tile_pool(name="hid", bufs=4))
        vgp = ctx.enter_context(tc.tile_pool(name="vg", bufs=2))
        wtep = ctx.enter_context(tc.tile_pool(name="wte", bufs=2))
        mrbp = ctx.enter_context(tc.tile_pool(name="mrb", bufs=2))
        rhsp = ctx.enter_context(tc.tile_pool(name="rhs", bufs=2))
        aggsp = ctx.enter_context(tc.tile_pool(name="aggs", bufs=2))
        ohp = ctx.enter_context(tc.tile_pool(name="oh", bufs=2))
        finp = ctx.enter_context(tc.tile_pool(name="fin", bufs=1))
        psh0p = ctx.enter_context(tc.tile_pool(name="psh0", bufs=2, space="PSUM"))
        psh1p = ctx.enter_context(tc.tile_pool(name="psh1", bufs=2, space="PSUM"))
        psmixp = ctx.enter_context(tc.tile_pool(name="psmix", bufs=2, space="PSUM"))
        psaggp = ctx.enter_context(tc.tile_pool(name="psagg", bufs=2, space="PSUM"))

        def _emit_all():
            # DMA order tuned so bond-0 compute of row-block 0 starts early
            bpk = constp.tile([128, BPKW], f32, tag="bpk", name="bpk")
            nc.sync.dma_start(out=bpk[:], in_=dram["bpk"][:])
            w0b0 = constp.tile([128, 256], bf, tag="w0b0", name="w0b0")
            nc.sync.dma_start(out=w0b0[:], in_=dram["w0b0"][:])
            xe0b = []
            for b in range(B):
                t = xep.tile([128, Lb[b]], bf, tag=f"xe0b{b}", name=f"xe0b{b}",
                             bufs=1)
                xe0b.append(t)
            nc.sync.dma_start(out=xe0b[0][:], in_=dram["xembT"][:, 0:Lb[0]])
            w1b0 = constp.tile([128, 256], bf, tag="w1b0", name="w1b0")
            nc.sync.dma_start(out=w1b0[:], in_=dram["w1b0"][:])
            w0r = constp.tile([128, 768], bf, tag="w0r", name="w0r")
            nc.sync.dma_start(out=w0r[:], in_=dram["w0r"][:])
            nc.sync.dma_start(out=xe0b[1][:],
                              in_=dram["xembT"][:, int(offs[1]):int(offs[2])])
            w1r = constp.tile([128, 768], bf, tag="w1r", name="w1r")
            nc.sync.dma_start(out=w1r[:], in_=dram["w1r"][:])
            wbf = constp.tile([128, WBFW], bf, tag="wbf", name="wbf")
            nc.sync.dma_start(out=wbf[:], in_=dram["wbf"][:])
            wpkt = constp.tile([128, WPKW], fr, tag="wpkt", name="wpkt")
            nc.sync.dma_start(out=wpkt[:], in_=dram["wpkt"][:])
            nc.sync.dma_start(out=xe0b[2][:],
                              in_=dram["xembT"][:, int(offs[2]):int(offs[3])])
            nc.sync.dma_start(out=xe0b[3][:],
                              in_=dram["xembT"][:, int(offs[3]):int(offs[4])])
            cpk = constp.tile([128, CPKW], f32, tag="cpk", name="cpk")
            nc.sync.dma_start(out=cpk[:], in_=dram["cpk"][:])
            b2esb = constp.tile([128, NTILE, H], f32, tag="b2e", name="b2e")
            nc.sync.dma_start(
                out=b2esb[:],
                in_=dram["b2e"][:].rearrange("p (t h) -> p t h", h=H))
            xes = [None]
            for rb in range(1, NRB):
                t = xep.tile([128, R], bf, tag="xe", name="xe")
                nc.sync.dma_start(
                    out=t[:], in_=dram["xembT"][:, rb * R:(rb + 1) * R])
                xes.append(t)

            def cp(nm):
                o, w = CPK[nm]
                return cpk[:, o:o + w]

            def bp(nm):
                o, w = BPK[nm]
                return bpk[:, o:o + w]

            def wp(nm):
                o, w = WPK[nm]
                return wpkt[:, o:o + w]

            def wb(nm):
                o, w = WBF[nm]
                return wbf[:, o:o + w]

            def w0_ap(b):
                return w0b0 if b == 0 else w0r[:, (b - 1) * 256:b * 256]

            def w1_ap(b):
                return w1b0 if b == 0 else w1r[:, (b - 1) * 256:b * 256]

            def xe_ap(rb, lo, ln):
                if rb == 0:
                    b = int(np.searchsorted(offs, lo, side="right") - 1)
                    return xe0b[b][:, lo - int(offs[b]): lo - int(offs[b]) + ln]
                return xes[rb][:, lo:lo + ln]

            psAs = {}

            def emit_head(rb, g, bonds, t0, tn):
                """MLP for this bond group; L3 -> edge-major psE; then Vg."""
                psE = psmixp.tile([128, tn * 4], f32, tag="mix",
                                  name=f"psE{g}")
                for b in bonds:
                    if Lb[b] == 0:
                        continue
                    for (cs, cl) in _chunks(Lb[b]):
                        lo = int(offs[b]) + cs
                        for pr in range(2):
                            i = b * 2 + pr
                            p0 = psh0p.tile([128, 512], f32, tag="h0", name="p0")
                            nc.tensor.matmul(
                                p0[:, :cl],
                                lhsT=w0_ap(b)[:, pr * 128:(pr + 1) * 128],
                                rhs=xe_ap(rb, lo, cl),
                                start=True, stop=True)
                            h0 = hidp.tile([128, 512], bf, tag="h0s", name="h0")
                            nc.scalar.activation(h0[:, :cl], p0[:, :cl], AF.Relu,
                                                 bias=bp("b0all")[:, i:i + 1])
                            p1 = psh1p.tile([128, 512], f32, tag="h1", name="p1")
                            nc.tensor.matmul(
                                p1[:, :cl],
                                lhsT=w1_ap(b)[:, pr * 128:(pr + 1) * 128],
                                rhs=h0[:, :cl],
                                start=True, stop=True)
                            dve_relu = (pr == 1 and b >= 2)
                            if dve_relu:
                                h1 = hidp.tile([128, 512], fr, tag="h1f",
                                               name="h1f")
                                nc.vector.tensor_scalar(
                                    out=h1[:, :cl], in0=p1[:, :cl],
                                    scalar1=bp("b1all")[:, i:i + 1],
                                    scalar2=0.0, op0=ALU.add, op1=ALU.max)
                                w2 = wp("w2fr")
                            else:
                                h1 = hidp.tile([128, 512], bf, tag="h1s",
                                               name="h1")
                                nc.scalar.activation(h1[:, :cl], p1[:, :cl],
                                                     AF.Relu,
                                                     bias=bp("b1all")[:, i:i + 1])
                                w2 = wb("w2all")
                            for j in range(cl // 128):
                                sl = (lo // 128) + j - t0
                                nc.tensor.matmul(
                                    psE[:, sl * 4 + pr * 2: sl * 4 + pr * 2 + 2],
                                    lhsT=h1[:, j * 128:(j + 1) * 128],
                                    rhs=w2[:, i * 2:(i + 1) * 2],
                                    start=True, stop=True)

                # V rows for this group's tiles
                vg = vgp.tile([128, tn, 64], fr, tag="vg", name="vg")
                for q0 in range(0, tn, 8):
                    qn = min(8, tn - q0)
                    pv = psaggp.tile([128, 512], f32, tag="agg", name="pv")
                    for q in range(q0, q0 + qn):
                        k = (q - q0) * 64
                        nc.tensor.matmul(pv[:, k:k + 64],
                                         lhsT=xe_ap(rb, (t0 + q) * 128, 128),
                                         rhs=wb("vwpad"),
                                         start=True, stop=True)
                    nc.vector.tensor_copy(
                        vg[:, q0:q0 + qn, :],
                        pv[:, :qn * 64].rearrange("p (t f) -> p t f", f=64))
                return vg, psE

            def emit_tail(rb, g, bonds, t0, tn, vg, psE):
                wte = wtep.tile([128, tn, 4], f32, tag="wte", name="wte")
                nc.vector.tensor_copy(wte[:],
                                      psE[:].rearrange("p (t f) -> p t f", f=4))
                nc.vector.tensor_tensor(
                    out=wte[:], in0=wte[:],
                    in1=b2esb[:, rb * TPB + t0: rb * TPB + t0 + tn, :],
                    op=ALU.add)
                wl = wtep.tile([128, tn, 4], f32, tag="wl", name="wl", bufs=1)
                nc.vector.tensor_scalar_mul(wl[:], wte[:], NEG)
                nc.vector.tensor_tensor(out=wte[:], in0=wte[:], in1=wl[:],
                                        op=ALU.max)
                nc.scalar.activation(wte[:], wte[:], AF.Exp)

                mrb = mrbp.tile([128, tn, 128], bf, tag="mrb", name="mrb")
                nc.vector.tensor_tensor(
                    out=mrb[:],
                    in0=cp("iota").unsqueeze(1).to_broadcast([128, tn, 128]),
                    in1=cp("srcrel")[:, rb * TPB + t0: rb * TPB + t0 + tn]
                        .unsqueeze(2).to_broadcast([128, tn, 128]),
                    op=ALU.is_equal)

                rhs = rhsp.tile([128, tn, 4 * 65], bf, tag="rhs", name="rhs")
                for h in range(H):
                    eng = nc.vector if h < 2 else nc.gpsimd
                    eng.tensor_tensor(
                        out=rhs[:, :, h * 65: h * 65 + 64],
                        in0=vg[:],
                        in1=wte[:, :, h:h + 1].to_broadcast([128, tn, 64]),
                        op=ALU.mult)
                    nc.vector.tensor_copy(rhs[:, :, h * 65 + 64: h * 65 + 65],
                                          wte[:, :, h:h + 1])

                if g == 0:
                    psAs[rb] = psaggp.tile([128, 4 * 65], f32, tag="agg",
                                           name="psA")
                psA = psAs[rb]
                for q in range(tn):
                    nc.tensor.matmul(psA[:],
                                     lhsT=mrb[:, q, :],
                                     rhs=rhs[:, q, :],
                                     start=(g == 0 and q == 0),
                                     stop=(g == NG - 1 and q == tn - 1))
                if g != NG - 1:
                    return

                aggsb = aggsp.tile([128, 4 * 65], f32, tag="aggsb", name="aggsb")
                nc.vector.tensor_copy(aggsb[:], psA[:])
                rz = ohp.tile([128, H], f32, tag="rz", name="rz", bufs=1)
                nc.vector.reciprocal(
                    rz[:], aggsb[:].rearrange("p (h z) -> p h z", z=65)[:, :, 64])
                oh = ohp.tile([128, H, 64], fr, tag="oh", name="oh")
                for h in range(H):
                    nc.vector.tensor_tensor(
                        out=oh[:, h, :],
                        in0=aggsb[:, h * 65: h * 65 + 64],
                        in1=rz[:, h:h + 1].to_broadcast([128, 64]),
                        op=ALU.mult)
                po = psaggp.tile([64, 512], fr, tag="agg", name="po")
                for h in range(H):
                    nc.tensor.transpose(out=po[:, h * 128:(h + 1) * 128],
                                        in_=oh[:, h, :],
                                        identity=wp("id128"))
                otrb = ohp.tile([64, H, 128], fr, tag="otrb", name="otrb")
                for h in range(H):
                    nc.vector.tensor_copy(otrb[:, h, :],
                                          po[:, h * 128:(h + 1) * 128])
                # project this row-block and ship it out immediately
                psP = psmixp.tile([64, 128], f32, tag="mix", name="psP")
                for h in range(H):
                    nc.tensor.matmul(psP[:],
                                     lhsT=wp("pw4")[0:64, h * 64:(h + 1) * 64],
                                     rhs=otrb[:, h, :],
                                     start=(h == 0), stop=(h == H - 1))
                outsb = finp.tile([64, 128], f32, tag="outsb", name="outsb",
                                  bufs=2)
                nc.vector.tensor_tensor(
                    out=outsb[:], in0=psP[:],
                    in1=cp("biascol")[0:64, :].to_broadcast([64, 128]),
                    op=ALU.add)
                nc.sync.dma_start(out=outT[:, rb * 128:(rb + 1) * 128],
                                  in_=outsb[:])

            # software pipeline at bond-group granularity: tail(u) follows
            # head(u+1), so every tail overlaps the next group's MLP stream
            units = [(rb, *grp) for rb in range(NRB) for grp in GRP]
            pend = None
            for u in units:
                rb, g, bonds, t0, tn = u
                hnd = emit_head(rb, g, bonds, t0, tn)
                if pend is not None:
                    (prb, pg, pbonds, pt0, ptn), ph = pend
                    emit_tail(prb, pg, pbonds, pt0, ptn, *ph)
                pend = (u, hnd)
            (prb, pg, pbonds, pt0, ptn), ph = pend
            emit_tail(prb, pg, pbonds, pt0, ptn, *ph)


        if loop:
            with tc.For_i(0, loop, 1):
                _emit_all()
        else:
            _emit_all()

    nc.compile()
    return nc


def _prepare(inputs):
    import ml_dtypes
    bf16 = ml_dtypes.bfloat16
    xembT, srcrel, bondslot, Lb, R = _host_prep(
        inputs["embeddings"], inputs["src"], inputs["dst"], inputs["bond"])
    wts = _weights_prep(inputs)
    b2 = np.asarray(inputs["b2"], np.float32)          # [B, H]
    b2e = b2[bondslot]                                  # [C, 128, NTILE, H]
    NTILE = (NRB * R) // 128
    f32 = np.float32

    cpk = np.zeros((128, NTILE + 128 + 1), f32)
    o = 0
    o_srcrel = o; o += NTILE
    cpk[:, o:o + 128] = wts["iota"]; o += 128
    cpk[0:64, o:o + 1] = wts["biascol"]; o += 1

    bpk = np.zeros((128, B * 2 + B * 2), f32)
    o = 0
    bpk[:, o:o + B * 2] = wts["b0all"]; o += B * 2
    bpk[:, o:o + B * 2] = wts["b1all"]; o += B * 2

    wbf = np.zeros((128, 64 + B * 2 * 2), bf16)
    o = 0
    wbf[:, o:o + 64] = wts["vwpad"].astype(bf16); o += 64
    wbf[:, o:o + B * 2 * 2] = wts["w2all"].astype(bf16); o += B * 2 * 2

    wpkt = np.zeros((128, H * 64 + 128 + B * 2 * 2), f32)
    o = 0
    wpkt[0:64, o:o + H * 64] = wts["pw4"]; o += H * 64
    wpkt[:, o:o + 128] = wts["id128"]; o += 128
    wpkt[:, o:o + B * 2 * 2] = wts["w2all"]; o += B * 2 * 2

    w0b0 = np.ascontiguousarray(wts["w0all"][:, 0:256]).astype(bf16)
    w0r = np.ascontiguousarray(wts["w0all"][:, 256:1024]).astype(bf16)
    w1b0 = np.ascontiguousarray(wts["w1all"][:, 0:256]).astype(bf16)
    w1r = np.ascontiguousarray(wts["w1all"][:, 256:1024]).astype(bf16)

    key = (tuple(Lb), R)
    if key not in _cache:
        _cache.clear()
        _cache[key] = _build_program(Lb, R)
    nc = _cache[key]
    in_maps = []
    for c in range(C):
        cpkc = cpk.copy()
        cpkc[:, o_srcrel:o_srcrel + NTILE] = srcrel[c]
        m = {"xembT": xembT[c].astype(bf16), "b2e": b2e[c].reshape(128, -1),
             "w0b0": w0b0, "w0r": w0r, "w1b0": w1b0, "w1r": w1r,
             "bpk": bpk, "wbf": wbf, "wpkt": wpkt, "cpk": cpkc}
        in_maps.append(m)
    return nc, in_maps


def kernel(**inputs):
    from concourse.bass_utils import run_bass_kernel_spmd

    nc, in_maps = _prepare(inputs)
    res = run_bass_kernel_spmd(nc, in_maps, list(range(C)))
    out = np.empty((N, D), np.float32)
    for c in range(C):
        out[c * RPC:(c + 1) * RPC] = res.results[c]["outT"].T
    return out


def benchmark(inputs, iters=10, warmup=2):
    """Time repeated executions of the compiled SPMD program with
    device-resident inputs (excludes compile and host<->device transfer)."""
    import time
    import jax
    from jax.experimental.shard_map import shard_map
    from jax.sharding import Mesh, PartitionSpec, NamedSharding
    from concourse import bass2jax as b2j
    from concourse import mybir

    nc, in_maps = _prepare(inputs)
    b2j.install_neuronx_cc_hook()
    partition_name = nc.partition_id_tensor.name if nc.partition_id_tensor else None
    in_names, out_names, out_avals, zero_outs = [], [], [], []
    for alloc in nc.m.functions[0].allocations:
        if not isinstance(alloc, mybir.MemoryLocationSet):
            continue
        name = alloc.memorylocations[0].name
        if alloc.kind == "ExternalInput":
            if name != partition_name:
                in_names.append(name)
        elif alloc.kind == "ExternalOutput":
            out_names.append(name)
            shape = tuple(alloc.tensor_shape)
            dtype = mybir.dt.np(alloc.dtype)
            out_avals.append(jax.core.ShapedArray(shape, dtype))
            zero_outs.append(np.zeros(shape, dtype))
    n_params = len(in_names)
    all_in = in_names + out_names + ([partition_name] if partition_name else [])
    donate = tuple(range(n_params, n_params + len(out_names)))

    def _body(*args):
        operands = list(args)
        if partition_name is not None:
            operands.append(b2j.partition_id_tensor())
        outs = b2j._bass_exec_p.bind(
            *operands, out_avals=tuple(out_avals), in_names=tuple(all_in),
            out_names=tuple(out_names), lowering_input_output_aliases=(),
            sim_require_finite=True, sim_require_nnan=True, nc=nc)
        return tuple(outs)

    devices = jax.devices()[:C]
    mesh = Mesh(np.asarray(devices), ("core",))
    in_specs = (PartitionSpec("core"),) * (n_params + len(out_names))
    out_specs = (PartitionSpec("core"),) * len(out_names)
    sharded = jax.jit(shard_map(_body, mesh=mesh, in_specs=in_specs,
                                out_specs=out_specs, check_rep=False),
                      donate_argnums=donate, keep_unused=True)
    sh = NamedSharding(mesh, PartitionSpec("core"))
    concat_in = [
        jax.device_put(
            np.concatenate([np.asarray(in_maps[c][n]) for c in range(C)], axis=0), sh)
        for n in in_names]

    times = []
    for it in range(warmup + iters):
        zs = [jax.device_put(np.zeros((C * z.shape[0], *z.shape[1:]), z.dtype), sh)
              for z in zero_outs]
        t0 = time.perf_counter()
        out = sharded(*concat_in, *zs)
        jax.block_until_ready(out)
        dt = time.perf_counter() - t0
        if it >= warmup:
            times.append(dt)
    print("bench times (ms):", [f"{t*1e3:.3f}" for t in times])
    return min(times) * 1e9


def benchmark_hw(inputs, k=512, iters=6, warmup=2, k_small=None):
    """Real-HW timing: run the whole per-core program k times inside one
    NEFF (tc.For_i) and wall-time it through the tunnel. If k_small is
    given, also times a k_small-loop NEFF and returns the difference
    quotient, which cancels the (~80ms) tunnel dispatch floor exactly."""
    if k_small:
        t_big = benchmark_hw(inputs, k=k, iters=iters, warmup=warmup)
        t_sml = benchmark_hw(inputs, k=k_small, iters=iters, warmup=warmup)
        return (t_big * k - t_sml * k_small) / (k - k_small)
    import time
    import jax
    from jax.experimental.shard_map import shard_map
    from jax.sharding import Mesh, PartitionSpec, NamedSharding
    from concourse import bass2jax as b2j
    from concourse import mybir

    xembT, srcrel, bondslot, Lb, R = _host_prep(
        inputs["embeddings"], inputs["src"], inputs["dst"], inputs["bond"])
    nc0, in_maps = _prepare(inputs)
    nc = _build_program(Lb, R, loop=k)

    b2j.install_neuronx_cc_hook()
    partition_name = nc.partition_id_tensor.name if nc.partition_id_tensor else None
    in_names, out_names, out_avals, zero_outs = [], [], [], []
    for alloc in nc.m.functions[0].allocations:
        if not isinstance(alloc, mybir.MemoryLocationSet):
            continue
        name = alloc.memorylocations[0].name
        if alloc.kind == "ExternalInput":
            if name != partition_name:
                in_names.append(name)
        elif alloc.kind == "ExternalOutput":
            out_names.append(name)
            shape = tuple(alloc.tensor_shape)
            dtype = mybir.dt.np(alloc.dtype)
            out_avals.append(jax.core.ShapedArray(shape, dtype))
            zero_outs.append(np.zeros(shape, dtype))
    n_params = len(in_names)
    all_in = in_names + out_names + ([partition_name] if partition_name else [])
    donate = tuple(range(n_params, n_params + len(out_names)))

    def _body(*args):
        operands = list(args)
        if partition_name is not None:
            operands.append(b2j.partition_id_tensor())
        outs = b2j._bass_exec_p.bind(
            *operands, out_avals=tuple(out_avals), in_names=tuple(all_in),
            out_names=tuple(out_names), lowering_input_output_aliases=(),
            sim_require_finite=True, sim_require_nnan=True, nc=nc)
        return tuple(outs)

    devices = jax.devices()[:C]
    mesh = Mesh(np.asarray(devices), ("core",))
    in_specs = (PartitionSpec("core"),) * (n_params + len(out_names))
    out_specs = (PartitionSpec("core"),) * len(out_names)
    sharded = jax.jit(shard_map(_body, mesh=mesh, in_specs=in_specs,
                                out_specs=out_specs, check_rep=False),
                      donate_argnums=donate, keep_unused=True)
    sh = NamedSharding(mesh, PartitionSpec("core"))
    concat_in = [
        jax.device_put(
            np.concatenate([np.asarray(in_maps[c][n]) for c in range(C)], axis=0),
            sh)
        for n in in_names]
    times = []
    for it in range(warmup + iters):
        zs = [jax.device_put(np.zeros((C * z.shape[0], *z.shape[1:]), z.dtype), sh)
              for z in zero_outs]
        t0 = time.perf_counter()
        out = sharded(*concat_in, *zs)
        jax.block_until_ready(out)
        dt = time.perf_counter() - t0
        if it >= warmup:
            times.append(dt)
    print("looped bench times (ms):", [f"{t*1e3:.2f}" for t in times])
    best = min(times)
    return best * 1e9 / k

